# revision 1
# baseline (speedup 1.0000x reference)
"""GraphSage 2-level mean-aggregator GNN on 8 trn2 NeuronCores.

Strategy (memory-bound problem: dif_mat_1 is [6000, 48000] f32 = 1.15 GB and
must stream through the chip exactly once):

  * Shard the level-1 diffusion matmul over its CONTRACTION dim (the 48000
    src selections): core k owns src columns [6000k, 6000(k+1)).  Each core
    streams its 144 MB dif_mat_1 shard (host-transposed to [src, dst] so the
    contraction dim lands on SBUF partitions) and computes a partial
    agg1^T = src_feat^T @ dif^T of shape [128, 6000].
  * Partials are AllReduce-summed in 6 slices of [128, 1024] so the
    collective overlaps the tail of the compute stream.
  * Every core then runs the small level-1 dense layer (relu(concat @ w1))
    redundantly, tiled so h1 chunks feed straight into the level-0
    aggregation as stationary matmul operands.
  * Level 0 is sharded by the 1024 target nodes (128 rows/core).  The
    h1[src_idx_0] gather AND the h1[dst_idx_0] selection are folded into one
    host-built matrix [difT0exp | one-hot-E] of shape [6000, 256], so the
    whole level collapses into one accumulated matmul over the h1 chunks.
  * Final dense + softmax per core on its 128 rows; host concatenates the
    eight [128, 40] outputs.  No device-side transposes or gathers needed.

All activations flow through the TensorEngine in natural layouts:
lhsT = [K, M] / rhs = [K, N] with the contraction dim always on partitions.
"""

import sys

import ml_dtypes
import numpy as np

sys.path.insert(0, "/opt/trn_rl_repo")

from concourse import bacc, bass_utils, mybir, tile

F32 = mybir.dt.float32
BF16 = mybir.dt.bfloat16

# Problem dims (hardcoded per spec)
N, F = 100000, 128
N1, D1, S1 = 60000, 6000, 48000
D0, S0 = 1024, 5000
H, C = 128, 40
NCORES = 8
CH = S1 // NCORES  # 6000 src columns per core
KT = 47            # contraction k-tiles of 128 (6016 = padded 6000)
KP = KT * 128      # 6016
JW = 512           # dst-block width for the big matmul
JB = 12            # dst blocks (6144 = padded 6000)
DP = JB * JW       # 6144
NAR = 6            # AllReduce slices (2 dst blocks each)
D0SH = D0 // NCORES  # 128 target rows per core

TRACE = False
TRACE_KW = {}
LAST = None  # last BassKernelResults (exec_time_ns when TRACE)

_nc = None


def _build(repeat=1):
    nc = bacc.Bacc(
        "TRN2",
        target_bir_lowering=False,
        debug=False,
        enable_asserts=False,
        num_devices=NCORES,
    )
    difT1 = nc.dram_tensor("difT1", [JB, KP, JW], BF16, kind="ExternalInput")
    sfeat = nc.dram_tensor("sfeat", [KP, F], BF16, kind="ExternalInput")
    dfT = nc.dram_tensor("dfT", [F, KP], BF16, kind="ExternalInput")
    d0e = nc.dram_tensor("d0e", [KP, 2 * H], BF16, kind="ExternalInput")
    w1t = nc.dram_tensor("w1t", [2 * F, H], BF16, kind="ExternalInput")
    w2t = nc.dram_tensor("w2t", [2 * H, H], BF16, kind="ExternalInput")
    wct = nc.dram_tensor("wct", [H, C], BF16, kind="ExternalInput")
    outd = nc.dram_tensor("out", [D0SH, C], F32, kind="ExternalOutput")

    rg = [list(range(NCORES))]
    relu = mybir.ActivationFunctionType.Relu

    with tile.TileContext(nc) as tc:
        with (
            tc.tile_pool(name="const", bufs=1) as constp,
            tc.tile_pool(name="stream", bufs=8) as streamp,
            tc.tile_pool(name="stage", bufs=3) as stagep,
            tc.tile_pool(name="h1p", bufs=3) as h1p,
            tc.tile_pool(name="ps1p", bufs=2, space="PSUM") as ps1p,
            tc.tile_pool(name="ps0p", bufs=1, space="PSUM") as ps0p,
            tc.tile_pool(name="ps2p", bufs=2, space="PSUM") as ps2p,
            tc.tile_pool(name="ps34p", bufs=1, space="PSUM") as ps34p,
            tc.tile_pool(name="dram", bufs=1, space="DRAM") as dramp,
        ):
            S_sb = constp.tile([128, KT, F], BF16, name="S_sb")
            dfT_sb = constp.tile([F, KP], BF16, name="dfT_sb")
            d0e_sb = constp.tile([128, KT, 2 * H], BF16, name="d0e_sb")
            w1_sb = constp.tile([128, 2, H], BF16, name="w1_sb")
            w2_sb = constp.tile([128, 2, H], BF16, name="w2_sb")
            wc_sb = constp.tile([H, C], BF16, name="wc_sb")
            ars_sb = constp.tile([F, DP], BF16, name="ars_sb")

            # constant loads (chunked to bound DMA descriptor counts)
            for h in range(0, KT, 12):
                hh = min(12, KT - h)
                nc.sync.dma_start(
                    S_sb[:, h : h + hh, :],
                    sfeat.ap()[h * 128 : (h + hh) * 128, :].rearrange(
                        "(c p) e -> p c e", p=128
                    ),
                )
                nc.sync.dma_start(
                    d0e_sb[:, h : h + hh, :],
                    d0e.ap()[h * 128 : (h + hh) * 128, :].rearrange(
                        "(c p) e -> p c e", p=128
                    ),
                )
            nc.sync.dma_start(dfT_sb[:], dfT.ap())
            nc.sync.dma_start(
                w1_sb[:], w1t.ap().rearrange("(c p) e -> p c e", p=128)
            )
            nc.sync.dma_start(
                w2_sb[:], w2t.ap().rearrange("(c p) e -> p c e", p=128)
            )
            nc.sync.dma_start(wc_sb[:], wct.ap())

            # ---- big streamed matmul: agg1T partial [128, 6144] ----
            # (`repeat` re-runs the whole pipeline for benchmark slope timing;
            # results are identical each rep so output is unchanged)
            for _rep in range(repeat):
              ar_ins, ar_outs = [], []
              for g in range(NAR):
                ai = dramp.tile([F, 2 * JW], BF16, name=f"ar_in{_rep}_{g}")
                ao = dramp.tile(
                    [F, 2 * JW], BF16, name=f"ar_out{_rep}_{g}", addr_space="Shared"
                )
                ar_ins.append(ai)
                ar_outs.append(ao)
              for j in range(JB):
                ps1 = ps1p.tile([F, JW], F32, tag="ps1")
                kt = 0
                for g in range(12):
                    gsz = 4 if g < 11 else 3
                    rt = streamp.tile([128, 4, JW], BF16, tag="rt")
                    eng = nc.sync
                    eng.dma_start(
                        rt[:, :gsz, :],
                        difT1.ap()[j, g * 512 : g * 512 + gsz * 128, :].rearrange(
                            "(c p) e -> p c e", p=128
                        ),
                    )
                    for c in range(gsz):
                        nc.tensor.matmul(
                            ps1[:],
                            S_sb[:, kt, :],
                            rt[:, c, :],
                            start=(kt == 0),
                            stop=(kt == KT - 1),
                        )
                        kt += 1
                st = stagep.tile([F, JW], BF16, tag="st")
                nc.vector.tensor_copy(st[:], ps1[:])
                g2, half = j // 2, j % 2
                nc.sync.dma_start(
                    ar_ins[g2][:, half * JW : (half + 1) * JW], st[:]
                )
                if half == 1:
                    nc.gpsimd.collective_compute(
                        "AllReduce",
                        mybir.AluOpType.add,
                        replica_groups=rg,
                        ins=[ar_ins[g2].opt()],
                        outs=[ar_outs[g2].opt()],
                    )
                    nc.sync.dma_start(
                        ars_sb[:, g2 * 2 * JW : (g2 + 1) * 2 * JW],
                        ar_outs[g2][:],
                    )

              # ---- level-1 dense + level-0 aggregation, fused per h1 chunk ----
              ps0 = ps0p.tile([H, 2 * H], F32, name="ps0")
              for t in range(KT):
                ps2 = ps2p.tile([128, H], F32, tag="ps2")
                nc.tensor.matmul(
                    ps2[:],
                    ars_sb[:, t * 128 : (t + 1) * 128],
                    w1_sb[:, 0, :],
                    start=True,
                    stop=False,
                )
                nc.tensor.matmul(
                    ps2[:],
                    dfT_sb[:, t * 128 : (t + 1) * 128],
                    w1_sb[:, 1, :],
                    start=False,
                    stop=True,
                )
                h1t = h1p.tile([128, H], BF16, tag="h1t")
                nc.scalar.activation(h1t[:], ps2[:], relu)
                nc.tensor.matmul(
                    ps0[:],
                    h1t[:],
                    d0e_sb[:, t, :],
                    start=(t == 0),
                    stop=(t == KT - 1),
                )

            # ---- level-0 dense + classifier + softmax ----
            c0_sb = constp.tile([H, 2 * H], BF16, name="c0_sb")
            nc.vector.tensor_copy(c0_sb[:], ps0[:])
            ps3 = ps34p.tile([H, D0SH], F32, tag="ps34")
            nc.tensor.matmul(
                ps3[:], w2_sb[:, 0, :], c0_sb[:, 0:H], start=True, stop=False
            )
            nc.tensor.matmul(
                ps3[:], w2_sb[:, 1, :], c0_sb[:, H : 2 * H], start=False, stop=True
            )
            h0T = constp.tile([H, D0SH], BF16, name="h0T")
            nc.scalar.activation(h0T[:], ps3[:], relu)
            ps4 = ps34p.tile([D0SH, C], F32, tag="ps34")
            nc.tensor.matmul(ps4[:], h0T[:], wc_sb[:], start=True, stop=True)

            mx = constp.tile([D0SH, 1], F32, name="mx")
            nc.vector.tensor_reduce(
                mx[:], ps4[:], axis=mybir.AxisListType.X, op=mybir.AluOpType.max
            )
            nmx = constp.tile([D0SH, 1], F32, name="nmx")
            nc.vector.tensor_scalar_mul(nmx[:], mx[:], -1.0)
            esb = constp.tile([D0SH, C], F32, name="esb")
            ssum = constp.tile([D0SH, 1], F32, name="ssum")
            nc.scalar.activation(
                esb[:],
                ps4[:],
                mybir.ActivationFunctionType.Exp,
                bias=nmx[:],
                accum_out=ssum[:],
            )
            rs = constp.tile([D0SH, 1], F32, name="rs")
            nc.vector.reciprocal(rs[:], ssum[:])
            osb = constp.tile([D0SH, C], F32, name="osb")
            nc.vector.tensor_scalar_mul(osb[:], esb[:], rs[:])
            nc.sync.dma_start(outd.ap(), osb[:])

    nc.compile()
    return nc


def _prep_in_maps(
    features,
    src_nodes,
    dst_idx_1,
    src_idx_1,
    dif_mat_1,
    dst_idx_0,
    src_idx_0,
    dif_mat_0,
    w1,
    w2,
    w_cls,
):
    f32 = np.float32
    bf16 = ml_dtypes.bfloat16
    features = np.asarray(features, f32)
    dif_mat_1 = np.asarray(dif_mat_1, f32)
    dif_mat_0 = np.asarray(dif_mat_0, f32)
    src_nodes = np.asarray(src_nodes)
    gsrc = src_nodes[np.asarray(src_idx_1)]  # [48000] rows into features
    gdst = src_nodes[np.asarray(dst_idx_1)]  # [6000]

    dfT = np.zeros((F, KP), f32)
    dfT[:, :D1] = features[gdst].T

    difT0exp = np.zeros((KP, D0), f32)
    np.add.at(difT0exp, np.asarray(src_idx_0), dif_mat_0.T)
    E = np.zeros((KP, D0), f32)
    E[np.asarray(dst_idx_0), np.arange(D0)] = 1.0

    w1c = np.ascontiguousarray(w1).astype(bf16)
    w2c = np.ascontiguousarray(w2).astype(bf16)
    wcc = np.ascontiguousarray(w_cls).astype(bf16)
    dfT16 = dfT.astype(bf16)

    full = D1 // JW  # 11 full dst blocks
    in_maps = []
    for k in range(NCORES):
        sl = slice(k * CH, (k + 1) * CH)
        At = np.ascontiguousarray(dif_mat_1[:, sl].T)  # [6000 src, 6000 dst]
        difT1 = np.zeros((JB, KP, JW), f32)
        difT1[:full, :CH, :] = At[:, : full * JW].reshape(CH, full, JW).transpose(
            1, 0, 2
        )
        difT1[full, :CH, : D1 - full * JW] = At[:, full * JW :]

        sfeat = np.zeros((KP, F), f32)
        sfeat[:CH] = features[gsrc[sl]]

        d0e = np.zeros((KP, 2 * H), f32)
        d0e[:, :H] = difT0exp[:, k * D0SH : (k + 1) * D0SH]
        d0e[:, H:] = E[:, k * D0SH : (k + 1) * D0SH]

        in_maps.append(
            {
                "difT1": difT1.astype(bf16),
                "sfeat": sfeat.astype(bf16),
                "dfT": dfT16,
                "d0e": d0e.astype(bf16),
                "w1t": w1c,
                "w2t": w2c,
                "wct": wcc,
            }
        )
    return in_maps


def kernel(**inputs):
    global _nc, LAST
    if _nc is None:
        _nc = _build()
    in_maps = _prep_in_maps(**inputs)
    res = bass_utils.run_bass_kernel_spmd(
        _nc,
        in_maps,
        core_ids=list(range(NCORES)),
        trace=TRACE,
        **TRACE_KW,
    )
    LAST = res
    out = np.concatenate([res.results[k]["out"] for k in range(NCORES)], axis=0)
    return out.astype(np.float32)



# revision 2
# speedup vs baseline: 1.6004x; 1.6004x over previous
"""GraphSage 2-level mean-aggregator GNN on 8 trn2 NeuronCores.

Strategy (memory-bound problem: dif_mat_1 is [6000, 48000] f32 = 1.15 GB and
must stream through the chip exactly once):

  * Shard the level-1 diffusion matmul over its CONTRACTION dim (the 48000
    src selections): core k owns src columns [6000k, 6000(k+1)).  The dif
    shard is quantized host-side to fp8-e3m4 (scaled by 2^19; the stationary
    src features are pre-scaled by 2^-19 so products come out exact) and
    packed partition-major so every stream DMA is a contiguous [128, x]
    slab with multi-KB per-partition lines — ~37 MB/core of HBM traffic.
  * Each slab feeds 47 accumulating matmuls (bf16 stationary x fp8 moving)
    producing a partial agg1^T = src_feat^T @ dif^T of shape [128, 6144].
  * Partials are AllReduce-summed in 6 slices of [128, 1024] bf16 so the
    collectives overlap the tail of the compute stream.
  * Every core then runs the small level-1 dense layer (relu(concat @ w1))
    redundantly, tiled so h1 chunks feed straight into the level-0
    aggregation as stationary matmul operands.
  * Level 0 is sharded by the 1024 target nodes (128 rows/core).  The
    h1[src_idx_0] gather AND the h1[dst_idx_0] selection are folded into one
    host-built matrix [difT0exp | one-hot-E] of shape [6016, 256], so the
    whole level collapses into one accumulated matmul over the h1 chunks.
  * Final dense + softmax per core on its 128 rows; host concatenates the
    eight [128, 40] outputs.  No device-side transposes or gathers needed.

All activations flow through the TensorEngine in natural layouts:
lhsT = [K, M] / rhs = [K, N] with the contraction dim always on partitions.
"""

import sys

import ml_dtypes
import numpy as np

sys.path.insert(0, "/opt/trn_rl_repo")

from concourse import bacc, bass_utils, mybir, tile

F32 = mybir.dt.float32
BF16 = mybir.dt.bfloat16
FP8E3 = mybir.dt.float8e3

# Problem dims (hardcoded per spec)
N, F = 100000, 128
N1, D1, S1 = 60000, 6000, 48000
D0, S0 = 1024, 5000
H, C = 128, 40
NCORES = 8
CH = S1 // NCORES  # 6000 src columns per core
KT = 47            # contraction k-tiles of 128 (6016 = padded 6000)
KP = KT * 128      # 6016
JW = 512           # dst-block width for the big matmul
JB = 12            # dst blocks (6144 = padded 6000)
DP = JB * JW       # 6144
NAR = 6            # AllReduce slices (2 dst blocks each)
D0SH = D0 // NCORES  # 128 target rows per core
SCALE = np.float32(2.0 ** 19)  # fp8 dif scale; src features carry 2^-19
HKT = 24           # k-tiles in the first half of a stream slab

TRACE = False
TRACE_KW = {}
LAST = None  # last BassKernelResults (exec_time_ns when TRACE)

_nc = None


def _build(repeat=1):
    nc = bacc.Bacc(
        "TRN2",
        target_bir_lowering=False,
        debug=False,
        enable_asserts=False,
        num_devices=NCORES,
    )
    difp = nc.dram_tensor("difp", [JB, 128, KT, JW], FP8E3, kind="ExternalInput")
    sfp = nc.dram_tensor("sfp", [128, KT, F], BF16, kind="ExternalInput")
    dfT = nc.dram_tensor("dfT", [F, KP], BF16, kind="ExternalInput")
    d0ep = nc.dram_tensor("d0ep", [128, KT, 2 * H], BF16, kind="ExternalInput")
    w1t = nc.dram_tensor("w1t", [2 * F, H], BF16, kind="ExternalInput")
    w2t = nc.dram_tensor("w2t", [2 * H, H], BF16, kind="ExternalInput")
    wct = nc.dram_tensor("wct", [H, C], BF16, kind="ExternalInput")
    outd = nc.dram_tensor("out", [D0SH, C], F32, kind="ExternalOutput")

    rg = [list(range(NCORES))]
    relu = mybir.ActivationFunctionType.Relu

    with tile.TileContext(nc) as tc:
        with (
            tc.tile_pool(name="const", bufs=1) as constp,
            tc.tile_pool(name="stream", bufs=3) as streamp,
            tc.tile_pool(name="stage", bufs=3) as stagep,
            tc.tile_pool(name="h1p", bufs=3) as h1p,
            tc.tile_pool(name="ps1p", bufs=2, space="PSUM") as ps1p,
            tc.tile_pool(name="ps0p", bufs=1, space="PSUM") as ps0p,
            tc.tile_pool(name="ps2p", bufs=2, space="PSUM") as ps2p,
            tc.tile_pool(name="ps34p", bufs=1, space="PSUM") as ps34p,
            tc.tile_pool(name="dram", bufs=1, space="DRAM") as dramp,
        ):
            S_sb = constp.tile([128, KT, F], BF16, name="S_sb")
            dfT_sb = constp.tile([F, KP], BF16, name="dfT_sb")
            d0e_sb = constp.tile([128, KT, 2 * H], BF16, name="d0e_sb")
            w1_sb = constp.tile([128, 2, H], BF16, name="w1_sb")
            w2_sb = constp.tile([128, 2, H], BF16, name="w2_sb")
            wc_sb = constp.tile([H, C], BF16, name="wc_sb")
            ars_sb = constp.tile([F, DP], BF16, name="ars_sb")

            # constant loads — all host-packed partition-major, one DMA each
            nc.sync.dma_start(S_sb[:], sfp.ap())
            nc.sync.dma_start(d0e_sb[:], d0ep.ap())
            nc.sync.dma_start(dfT_sb[:], dfT.ap())
            nc.sync.dma_start(
                w1_sb[:], w1t.ap().rearrange("(c p) e -> p c e", p=128)
            )
            nc.sync.dma_start(
                w2_sb[:], w2t.ap().rearrange("(c p) e -> p c e", p=128)
            )
            nc.sync.dma_start(wc_sb[:], wct.ap())

            # ---- big streamed matmul: agg1T partial [128, 6144] ----
            # (`repeat` re-runs the whole pipeline for benchmark slope timing;
            # results are identical each rep so output is unchanged)
            for _rep in range(repeat):
              ar_ins, ar_outs = [], []
              for g in range(NAR):
                ai = dramp.tile([F, 2 * JW], BF16, name=f"ar_in{_rep}_{g}")
                ao = dramp.tile(
                    [F, 2 * JW], BF16, name=f"ar_out{_rep}_{g}", addr_space="Shared"
                )
                ar_ins.append(ai)
                ar_outs.append(ao)
              for j in range(JB):
                # two half-slab DMAs per dst block so the first matmul only
                # waits on half the 3 MB stream transfer
                ra = streamp.tile([128, HKT, JW], FP8E3, tag="ra")
                rb = streamp.tile([128, KT - HKT, JW], FP8E3, tag="rb")
                nc.sync.dma_start(ra[:], difp.ap()[j, :, 0:HKT, :])
                nc.sync.dma_start(rb[:], difp.ap()[j, :, HKT:KT, :])
                ps1 = ps1p.tile([F, JW], F32, tag="ps1")
                for kt in range(KT):
                    rt = ra[:, kt, :] if kt < HKT else rb[:, kt - HKT, :]
                    nc.tensor.matmul(
                        ps1[:],
                        S_sb[:, kt, :],
                        rt,
                        start=(kt == 0),
                        stop=(kt == KT - 1),
                    )
                st = stagep.tile([F, JW], BF16, tag="st")
                nc.vector.tensor_copy(st[:], ps1[:])
                g2, half = j // 2, j % 2
                nc.sync.dma_start(
                    ar_ins[g2][:, half * JW : (half + 1) * JW], st[:]
                )
                if half == 1:
                    nc.gpsimd.collective_compute(
                        "AllReduce",
                        mybir.AluOpType.add,
                        replica_groups=rg,
                        ins=[ar_ins[g2].opt()],
                        outs=[ar_outs[g2].opt()],
                    )
                    nc.sync.dma_start(
                        ars_sb[:, g2 * 2 * JW : (g2 + 1) * 2 * JW],
                        ar_outs[g2][:],
                    )

              # ---- level-1 dense + level-0 aggregation, fused per h1 chunk ----
              ps0 = ps0p.tile([H, 2 * H], F32, name="ps0")
              for t in range(KT):
                ps2 = ps2p.tile([128, H], F32, tag="ps2")
                nc.tensor.matmul(
                    ps2[:],
                    ars_sb[:, t * 128 : (t + 1) * 128],
                    w1_sb[:, 0, :],
                    start=True,
                    stop=False,
                )
                nc.tensor.matmul(
                    ps2[:],
                    dfT_sb[:, t * 128 : (t + 1) * 128],
                    w1_sb[:, 1, :],
                    start=False,
                    stop=True,
                )
                h1t = h1p.tile([128, H], BF16, tag="h1t")
                nc.scalar.activation(h1t[:], ps2[:], relu)
                nc.tensor.matmul(
                    ps0[:],
                    h1t[:],
                    d0e_sb[:, t, :],
                    start=(t == 0),
                    stop=(t == KT - 1),
                )

            # ---- level-0 dense + classifier + softmax ----
            c0_sb = constp.tile([H, 2 * H], BF16, name="c0_sb")
            nc.vector.tensor_copy(c0_sb[:], ps0[:])
            ps3 = ps34p.tile([H, D0SH], F32, tag="ps34")
            nc.tensor.matmul(
                ps3[:], w2_sb[:, 0, :], c0_sb[:, 0:H], start=True, stop=False
            )
            nc.tensor.matmul(
                ps3[:], w2_sb[:, 1, :], c0_sb[:, H : 2 * H], start=False, stop=True
            )
            h0T = constp.tile([H, D0SH], BF16, name="h0T")
            nc.scalar.activation(h0T[:], ps3[:], relu)
            ps4 = ps34p.tile([D0SH, C], F32, tag="ps34")
            nc.tensor.matmul(ps4[:], h0T[:], wc_sb[:], start=True, stop=True)

            mx = constp.tile([D0SH, 1], F32, name="mx")
            nc.vector.tensor_reduce(
                mx[:], ps4[:], axis=mybir.AxisListType.X, op=mybir.AluOpType.max
            )
            nmx = constp.tile([D0SH, 1], F32, name="nmx")
            nc.vector.tensor_scalar_mul(nmx[:], mx[:], -1.0)
            esb = constp.tile([D0SH, C], F32, name="esb")
            ssum = constp.tile([D0SH, 1], F32, name="ssum")
            nc.scalar.activation(
                esb[:],
                ps4[:],
                mybir.ActivationFunctionType.Exp,
                bias=nmx[:],
                accum_out=ssum[:],
            )
            rs = constp.tile([D0SH, 1], F32, name="rs")
            nc.vector.reciprocal(rs[:], ssum[:])
            osb = constp.tile([D0SH, C], F32, name="osb")
            nc.vector.tensor_scalar_mul(osb[:], esb[:], rs[:])
            nc.sync.dma_start(outd.ap(), osb[:])

    nc.compile()
    return nc


def _prep_in_maps(
    features,
    src_nodes,
    dst_idx_1,
    src_idx_1,
    dif_mat_1,
    dst_idx_0,
    src_idx_0,
    dif_mat_0,
    w1,
    w2,
    w_cls,
):
    f32 = np.float32
    bf16 = ml_dtypes.bfloat16
    fp8 = ml_dtypes.float8_e3m4
    features = np.asarray(features, f32)
    dif_mat_1 = np.asarray(dif_mat_1, f32)
    dif_mat_0 = np.asarray(dif_mat_0, f32)
    src_nodes = np.asarray(src_nodes)
    gsrc = src_nodes[np.asarray(src_idx_1)]  # [48000] rows into features
    gdst = src_nodes[np.asarray(dst_idx_1)]  # [6000]

    dfT = np.zeros((F, KP), f32)
    dfT[:, :D1] = features[gdst].T

    difT0exp = np.zeros((KP, D0), f32)
    np.add.at(difT0exp, np.asarray(src_idx_0), dif_mat_0.T)
    E = np.zeros((KP, D0), f32)
    E[np.asarray(dst_idx_0), np.arange(D0)] = 1.0

    w1c = np.ascontiguousarray(w1).astype(bf16)
    w2c = np.ascontiguousarray(w2).astype(bf16)
    wcc = np.ascontiguousarray(w_cls).astype(bf16)
    dfT16 = dfT.astype(bf16)

    in_maps = []
    for k in range(NCORES):
        sl = slice(k * CH, (k + 1) * CH)
        # fp8 stream, packed [JB, 128, KT, JW]:
        #   difp[j, p, kt, e] = dif[src = kt*128+p, dst = j*512+e] * SCALE
        P = np.zeros((KP, DP), f32)
        P[:CH, :D1] = dif_mat_1[:, sl].T
        Q = (P * SCALE).astype(fp8)
        difp = np.ascontiguousarray(
            Q.reshape(KT, 128, JB, JW).transpose(2, 1, 0, 3)
        )

        # src features, scaled by 2^-19, packed [128, KT, F]
        sf = np.zeros((KP, F), f32)
        sf[:CH] = features[gsrc[sl]]
        sfp = np.ascontiguousarray(
            (sf / SCALE).astype(bf16).reshape(KT, 128, F).transpose(1, 0, 2)
        )

        # [difT0exp | E] columns for this core's targets, packed [128, KT, 2H]
        d0e = np.zeros((KP, 2 * H), f32)
        d0e[:, :H] = difT0exp[:, k * D0SH : (k + 1) * D0SH]
        d0e[:, H:] = E[:, k * D0SH : (k + 1) * D0SH]
        d0ep = np.ascontiguousarray(
            d0e.astype(bf16).reshape(KT, 128, 2 * H).transpose(1, 0, 2)
        )

        in_maps.append(
            {
                "difp": difp,
                "sfp": sfp,
                "dfT": dfT16,
                "d0ep": d0ep,
                "w1t": w1c,
                "w2t": w2c,
                "wct": wcc,
            }
        )
    return in_maps


def kernel(**inputs):
    global _nc, LAST
    if _nc is None:
        _nc = _build()
    in_maps = _prep_in_maps(**inputs)
    res = bass_utils.run_bass_kernel_spmd(
        _nc,
        in_maps,
        core_ids=list(range(NCORES)),
        trace=TRACE,
        **TRACE_KW,
    )
    LAST = res
    out = np.concatenate([res.results[k]["out"] for k in range(NCORES)], axis=0)
    return out.astype(np.float32)


# revision 3
# speedup vs baseline: 1.6097x; 1.0058x over previous
"""GraphSage 2-level mean-aggregator GNN on 8 trn2 NeuronCores.

Strategy (memory-bound problem: dif_mat_1 is [6000, 48000] f32 = 1.15 GB and
must stream through the chip exactly once):

  * Shard the level-1 diffusion matmul over its CONTRACTION dim (the 48000
    src selections): core k owns src columns [6000k, 6000(k+1)).  The dif
    shard AND the stationary src features are quantized host-side to
    fp8-e4m3 (dif scaled by 2^23; the product is descaled on the PSUM
    copy-out), packed partition-major so every stream DMA is a contiguous
    [128, x] slab with 12 KB per-partition lines — ~37 MB/core of traffic.
  * The stream matmuls run in DoubleRow perf mode (2 contraction k-tiles
    per instruction, 2x PE throughput) producing a partial
    agg1^T = src_feat^T @ dif^T of shape [128, 6144].  Slab DMAs alternate
    between the sync and scalar HWDGE rings to keep all 16 SDMA engines fed.
  * Partials are AllReduce-summed in 6 slices of [128, 1024] bf16 so the
    collectives overlap the tail of the compute stream.
  * Every core then runs the small level-1 dense layer (relu(concat @ w1))
    redundantly, tiled so h1 chunks feed straight into the level-0
    aggregation as stationary matmul operands.
  * Level 0 is sharded by the 1024 target nodes (128 rows/core).  The
    h1[src_idx_0] gather AND the h1[dst_idx_0] selection are folded into one
    host-built matrix [difT0exp | one-hot-E] of shape [6016, 256] (fp8-e4m3
    scaled by 2^7, descaled on the c0 copy), so the whole level collapses
    into one accumulated matmul over the h1 chunks.
  * Final dense + softmax per core on its 128 rows; host concatenates the
    eight [128, 40] outputs.  No device-side transposes or gathers needed.

All activations flow through the TensorEngine in natural layouts:
lhsT = [K, M] / rhs = [K, N] with the contraction dim always on partitions.
"""

import sys

import ml_dtypes
import numpy as np

sys.path.insert(0, "/opt/trn_rl_repo")

from concourse import bacc, bass_utils, mybir, tile

F32 = mybir.dt.float32
BF16 = mybir.dt.bfloat16
FP8E4 = mybir.dt.float8e4

# Problem dims (hardcoded per spec)
N, F = 100000, 128
N1, D1, S1 = 60000, 6000, 48000
D0, S0 = 1024, 5000
H, C = 128, 40
NCORES = 8
CH = S1 // NCORES  # 6000 src columns per core
KT = 47            # contraction k-tiles of 128 (6016 = padded 6000)
KP = KT * 128      # 6016
JW = 512           # dst-block width for the big matmul
JB = 12            # dst blocks (6144 = padded 6000)
DP = JB * JW       # 6144
NAR = 6            # AllReduce slices (2 dst blocks each)
D0SH = D0 // NCORES  # 128 target rows per core
SCALE = np.float32(2.0 ** 23)   # fp8 dif scale; descaled on PSUM copy-out
ISCALE = float(1.0 / SCALE)
SC0 = np.float32(2.0 ** 7)      # fp8 d0e scale; descaled on c0 copy
ISC0 = float(1.0 / SC0)
HKT = 24           # k-tiles in the first half of a stream slab

TRACE = False
TRACE_KW = {}
LAST = None  # last BassKernelResults (exec_time_ns when TRACE)

_nc = None


def _build(repeat=1):
    nc = bacc.Bacc(
        "TRN2",
        target_bir_lowering=False,
        debug=False,
        enable_asserts=False,
        num_devices=NCORES,
    )
    difp = nc.dram_tensor("difp", [JB, 128, KT, JW], FP8E4, kind="ExternalInput")
    sfp = nc.dram_tensor("sfp", [128, KT, F], FP8E4, kind="ExternalInput")
    dfT = nc.dram_tensor("dfT", [F, KP], BF16, kind="ExternalInput")
    d0ep = nc.dram_tensor("d0ep", [128, KT, 2 * H], FP8E4, kind="ExternalInput")
    w1t = nc.dram_tensor("w1t", [2 * F, H], BF16, kind="ExternalInput")
    w2t = nc.dram_tensor("w2t", [2 * H, H], BF16, kind="ExternalInput")
    wct = nc.dram_tensor("wct", [H, C], BF16, kind="ExternalInput")
    outd = nc.dram_tensor("out", [D0SH, C], F32, kind="ExternalOutput")

    rg = [list(range(NCORES))]
    relu = mybir.ActivationFunctionType.Relu
    DR = mybir.MatmulPerfMode.DoubleRow

    with tile.TileContext(nc) as tc:
        with (
            tc.tile_pool(name="const", bufs=1) as constp,
            tc.tile_pool(name="stream", bufs=6) as streamp,
            tc.tile_pool(name="stage", bufs=3) as stagep,
            tc.tile_pool(name="h1p", bufs=3) as h1p,
            tc.tile_pool(name="ps1p", bufs=2, space="PSUM") as ps1p,
            tc.tile_pool(name="ps0p", bufs=1, space="PSUM") as ps0p,
            tc.tile_pool(name="ps2p", bufs=2, space="PSUM") as ps2p,
            tc.tile_pool(name="ps34p", bufs=1, space="PSUM") as ps34p,
            tc.tile_pool(name="dram", bufs=1, space="DRAM") as dramp,
        ):
            S_sb = constp.tile([128, KT, F], FP8E4, name="S_sb")
            dfT_sb = constp.tile([F, KP], BF16, name="dfT_sb")
            d0e_sb = constp.tile([128, KT, 2 * H], FP8E4, name="d0e_sb")
            w1_sb = constp.tile([128, 2, H], BF16, name="w1_sb")
            w2_sb = constp.tile([128, 2, H], BF16, name="w2_sb")
            wc_sb = constp.tile([H, C], BF16, name="wc_sb")
            ars_sb = constp.tile([F, DP], BF16, name="ars_sb")

            # stream-critical constants first on the stream rings, the rest
            # on the gpsimd (SWDGE) ring so they don't delay the first slab
            nc.sync.dma_start(S_sb[:], sfp.ap())
            nc.gpsimd.dma_start(d0e_sb[:], d0ep.ap())
            nc.gpsimd.dma_start(dfT_sb[:], dfT.ap())
            nc.gpsimd.dma_start(
                w1_sb[:], w1t.ap().rearrange("(c p) e -> p c e", p=128)
            )
            nc.gpsimd.dma_start(
                w2_sb[:], w2t.ap().rearrange("(c p) e -> p c e", p=128)
            )
            nc.gpsimd.dma_start(wc_sb[:], wct.ap())

            # ---- big streamed matmul: agg1T partial [128, 6144] ----
            # (`repeat` re-runs the whole pipeline for benchmark slope timing;
            # results are identical each rep so output is unchanged)
            for _rep in range(repeat):
              ar_ins, ar_outs = [], []
              for g in range(NAR):
                ai = dramp.tile([F, 2 * JW], BF16, name=f"ar_in{_rep}_{g}")
                ao = dramp.tile(
                    [F, 2 * JW], BF16, name=f"ar_out{_rep}_{g}", addr_space="Shared"
                )
                ar_ins.append(ai)
                ar_outs.append(ao)
              for j in range(JB):
                # two half-slab DMAs per dst block on the two HWDGE rings so
                # the first matmul only waits on half the 3 MB transfer
                ra = streamp.tile([128, HKT, JW], FP8E4, tag="ra")
                rb = streamp.tile([128, KT - HKT, JW], FP8E4, tag="rb")
                nc.sync.dma_start(ra[:], difp.ap()[j, :, 0:HKT, :])
                nc.scalar.dma_start(rb[:], difp.ap()[j, :, HKT:KT, :])
                ps1 = ps1p.tile([F, JW], F32, tag="ps1")
                for p in range(HKT // 2):
                    nc.tensor.matmul(
                        ps1[:],
                        S_sb[:, 2 * p : 2 * p + 2, :],
                        ra[:, 2 * p : 2 * p + 2, :],
                        start=(p == 0),
                        stop=False,
                        perf_mode=DR,
                    )
                for p in range(HKT // 2, KT // 2):
                    q = 2 * p - HKT
                    nc.tensor.matmul(
                        ps1[:],
                        S_sb[:, 2 * p : 2 * p + 2, :],
                        rb[:, q : q + 2, :],
                        start=False,
                        stop=False,
                        perf_mode=DR,
                    )
                nc.tensor.matmul(
                    ps1[:],
                    S_sb[:, KT - 1, :],
                    rb[:, KT - 1 - HKT, :],
                    start=False,
                    stop=True,
                )
                st = stagep.tile([F, JW], BF16, tag="st")
                nc.vector.tensor_scalar_mul(st[:], ps1[:], ISCALE)
                g2, half = j // 2, j % 2
                nc.gpsimd.dma_start(
                    ar_ins[g2][:, half * JW : (half + 1) * JW], st[:]
                )
                if half == 1:
                    nc.gpsimd.collective_compute(
                        "AllReduce",
                        mybir.AluOpType.add,
                        replica_groups=rg,
                        ins=[ar_ins[g2].opt()],
                        outs=[ar_outs[g2].opt()],
                    )
                    nc.gpsimd.dma_start(
                        ars_sb[:, g2 * 2 * JW : (g2 + 1) * 2 * JW],
                        ar_outs[g2][:],
                    )

              # ---- level-1 dense + level-0 aggregation, fused per h1 chunk ----
              ps0 = ps0p.tile([H, 2 * H], F32, name="ps0")
              for t in range(KT):
                ps2 = ps2p.tile([128, H], F32, tag="ps2")
                nc.tensor.matmul(
                    ps2[:],
                    ars_sb[:, t * 128 : (t + 1) * 128],
                    w1_sb[:, 0, :],
                    start=True,
                    stop=False,
                )
                nc.tensor.matmul(
                    ps2[:],
                    dfT_sb[:, t * 128 : (t + 1) * 128],
                    w1_sb[:, 1, :],
                    start=False,
                    stop=True,
                )
                h1t = h1p.tile([128, H], BF16, tag="h1t")
                nc.scalar.activation(h1t[:], ps2[:], relu)
                nc.tensor.matmul(
                    ps0[:],
                    h1t[:],
                    d0e_sb[:, t, :],
                    start=(t == 0),
                    stop=(t == KT - 1),
                )

            # ---- level-0 dense + classifier + softmax ----
            c0_sb = constp.tile([H, 2 * H], BF16, name="c0_sb")
            nc.vector.tensor_scalar_mul(c0_sb[:], ps0[:], ISC0)
            ps3 = ps34p.tile([H, D0SH], F32, tag="ps34")
            nc.tensor.matmul(
                ps3[:], w2_sb[:, 0, :], c0_sb[:, 0:H], start=True, stop=False
            )
            nc.tensor.matmul(
                ps3[:], w2_sb[:, 1, :], c0_sb[:, H : 2 * H], start=False, stop=True
            )
            h0T = constp.tile([H, D0SH], BF16, name="h0T")
            nc.scalar.activation(h0T[:], ps3[:], relu)
            ps4 = ps34p.tile([D0SH, C], F32, tag="ps34")
            nc.tensor.matmul(ps4[:], h0T[:], wc_sb[:], start=True, stop=True)

            mx = constp.tile([D0SH, 1], F32, name="mx")
            nc.vector.tensor_reduce(
                mx[:], ps4[:], axis=mybir.AxisListType.X, op=mybir.AluOpType.max
            )
            nmx = constp.tile([D0SH, 1], F32, name="nmx")
            nc.vector.tensor_scalar_mul(nmx[:], mx[:], -1.0)
            esb = constp.tile([D0SH, C], F32, name="esb")
            ssum = constp.tile([D0SH, 1], F32, name="ssum")
            nc.scalar.activation(
                esb[:],
                ps4[:],
                mybir.ActivationFunctionType.Exp,
                bias=nmx[:],
                accum_out=ssum[:],
            )
            rs = constp.tile([D0SH, 1], F32, name="rs")
            nc.vector.reciprocal(rs[:], ssum[:])
            osb = constp.tile([D0SH, C], F32, name="osb")
            nc.vector.tensor_scalar_mul(osb[:], esb[:], rs[:])
            nc.sync.dma_start(outd.ap(), osb[:])

    nc.compile()
    return nc


def _prep_in_maps(
    features,
    src_nodes,
    dst_idx_1,
    src_idx_1,
    dif_mat_1,
    dst_idx_0,
    src_idx_0,
    dif_mat_0,
    w1,
    w2,
    w_cls,
):
    f32 = np.float32
    bf16 = ml_dtypes.bfloat16
    fp8 = ml_dtypes.float8_e4m3
    features = np.asarray(features, f32)
    dif_mat_1 = np.asarray(dif_mat_1, f32)
    dif_mat_0 = np.asarray(dif_mat_0, f32)
    src_nodes = np.asarray(src_nodes)
    gsrc = src_nodes[np.asarray(src_idx_1)]  # [48000] rows into features
    gdst = src_nodes[np.asarray(dst_idx_1)]  # [6000]

    dfT = np.zeros((F, KP), f32)
    dfT[:, :D1] = features[gdst].T

    difT0exp = np.zeros((KP, D0), f32)
    np.add.at(difT0exp, np.asarray(src_idx_0), dif_mat_0.T)
    E = np.zeros((KP, D0), f32)
    E[np.asarray(dst_idx_0), np.arange(D0)] = 1.0

    w1c = np.ascontiguousarray(w1).astype(bf16)
    w2c = np.ascontiguousarray(w2).astype(bf16)
    wcc = np.ascontiguousarray(w_cls).astype(bf16)
    dfT16 = dfT.astype(bf16)

    in_maps = []
    for k in range(NCORES):
        sl = slice(k * CH, (k + 1) * CH)
        # fp8 stream, packed [JB, 128, KT, JW]:
        #   difp[j, p, kt, e] = dif[src = kt*128+p, dst = j*512+e] * SCALE
        P = np.zeros((KP, DP), f32)
        P[:CH, :D1] = dif_mat_1[:, sl].T
        Q = (P * SCALE).astype(fp8)
        difp = np.ascontiguousarray(
            Q.reshape(KT, 128, JB, JW).transpose(2, 1, 0, 3)
        )

        # src features (unscaled fp8), packed [128, KT, F]
        sf = np.zeros((KP, F), f32)
        sf[:CH] = features[gsrc[sl]]
        sfp = np.ascontiguousarray(
            sf.astype(fp8).reshape(KT, 128, F).transpose(1, 0, 2)
        )

        # [difT0exp | E] columns for this core's targets, packed [128, KT, 2H]
        d0e = np.zeros((KP, 2 * H), f32)
        d0e[:, :H] = difT0exp[:, k * D0SH : (k + 1) * D0SH]
        d0e[:, H:] = E[:, k * D0SH : (k + 1) * D0SH]
        d0ep = np.ascontiguousarray(
            (d0e * SC0).astype(fp8).reshape(KT, 128, 2 * H).transpose(1, 0, 2)
        )

        in_maps.append(
            {
                "difp": difp,
                "sfp": sfp,
                "dfT": dfT16,
                "d0ep": d0ep,
                "w1t": w1c,
                "w2t": w2c,
                "wct": wcc,
            }
        )
    return in_maps


def kernel(**inputs):
    global _nc, LAST
    if _nc is None:
        _nc = _build()
    in_maps = _prep_in_maps(**inputs)
    res = bass_utils.run_bass_kernel_spmd(
        _nc,
        in_maps,
        core_ids=list(range(NCORES)),
        trace=TRACE,
        **TRACE_KW,
    )
    LAST = res
    out = np.concatenate([res.results[k]["out"] for k in range(NCORES)], axis=0)
    return out.astype(np.float32)


# revision 4
# speedup vs baseline: 1.8559x; 1.1530x over previous
"""GraphSage 2-level mean-aggregator GNN on 8 trn2 NeuronCores.

Strategy (memory-bound problem: dif_mat_1 is [6000, 48000] f32 = 1.15 GB and
must stream through the chip exactly once):

  * Shard the level-1 diffusion matmul over its CONTRACTION dim (the 48000
    src selections): core k owns src columns [6000k, 6000(k+1)).  The dif
    shard AND the stationary src features are quantized host-side to
    fp8-e4m3 (dif scaled by 2^23; the product is descaled on the PSUM
    copy-out), packed partition-major so every stream DMA is a contiguous
    [128, x] slab with ~12 KB per-partition lines — ~37 MB/core of traffic.
  * The stream matmuls run in DoubleRow perf mode (2 contraction k-tiles
    per instruction, 2x PE throughput) producing a partial
    agg1^T = src_feat^T @ dif^T of shape [128, 6000].  Slab DMAs alternate
    between the sync and scalar HWDGE rings (nothing else rides those rings
    mid-stream, so the stream is never head-of-line blocked); partial
    copies out to DRAM ride the gpsimd ring.
  * Partials are AllReduce-summed in 3 slices of ~[128, 2048] bf16.  The
    collective triggers sit on the gpsimd ring right behind the partial
    writes, so slices 0/1 overlap the stream and only slice 2's ~15 us
    latency is exposed at the tail.  The summed slices are fetched on the
    sync ring after the stream.
  * Every core then runs the small level-1 dense layer (relu(concat @ w1))
    redundantly, tiled so h1 chunks feed straight into the level-0
    aggregation as stationary matmul operands.
  * Level 0 is sharded by the 1024 target nodes (128 rows/core).  The
    h1[src_idx_0] gather AND the h1[dst_idx_0] selection are folded into one
    host-built matrix [difT0exp | one-hot-E] of shape [6016, 256] (fp8-e4m3
    scaled by 2^7, descaled on the c0 copy), so the whole level collapses
    into one accumulated matmul over the h1 chunks.
  * Final dense + softmax per core on its 128 rows; host concatenates the
    eight [128, 40] outputs.  No device-side transposes or gathers needed.

All activations flow through the TensorEngine in natural layouts:
lhsT = [K, M] / rhs = [K, N] with the contraction dim always on partitions.
"""

import sys

import ml_dtypes
import numpy as np

sys.path.insert(0, "/opt/trn_rl_repo")

from concourse import bacc, bass_utils, mybir, tile

F32 = mybir.dt.float32
BF16 = mybir.dt.bfloat16
FP8E4 = mybir.dt.float8e4

# Problem dims (hardcoded per spec)
N, F = 100000, 128
N1, D1, S1 = 60000, 6000, 48000
D0, S0 = 1024, 5000
H, C = 128, 40
NCORES = 8
CH = S1 // NCORES  # 6000 src columns per core
KT = 47            # contraction k-tiles of 128 (6016 = padded 6000)
KP = KT * 128      # 6016
JW = 512           # dst-block width for the big matmul
JB = 12            # dst blocks; the last is 368 wide (11*512 + 368 = 6000)
JWL = D1 - (JB - 1) * JW  # 368
D0SH = D0 // NCORES  # 128 target rows per core
SCALE = np.float32(2.0 ** 23)   # fp8 dif scale; descaled on PSUM copy-out
ISCALE = float(1.0 / SCALE)
SC0 = np.float32(2.0 ** 7)      # fp8 d0e scale; descaled on c0 copy
ISC0 = float(1.0 / SC0)
HKT = 24           # k-tiles in the first half of a stream slab
# AllReduce groups: (first j, last j exclusive, dst-col offset, width)
ARG = [(0, 4, 0, 2048), (4, 8, 2048, 2048), (8, 12, 4096, 1904)]

TRACE = False
TRACE_KW = {}
LAST = None  # last BassKernelResults (exec_time_ns when TRACE)

_nc = None


def _build(repeat=1):
    nc = bacc.Bacc(
        "TRN2",
        target_bir_lowering=False,
        debug=False,
        enable_asserts=False,
        num_devices=NCORES,
    )
    difp = nc.dram_tensor(
        "difp", [JB - 1, 128, KT, JW], FP8E4, kind="ExternalInput"
    )
    difl = nc.dram_tensor("difl", [128, KT, JWL], FP8E4, kind="ExternalInput")
    sfp = nc.dram_tensor("sfp", [128, KT, F], FP8E4, kind="ExternalInput")
    dfT = nc.dram_tensor("dfT", [F, KP], BF16, kind="ExternalInput")
    d0ep = nc.dram_tensor("d0ep", [128, KT, 2 * H], FP8E4, kind="ExternalInput")
    w1t = nc.dram_tensor("w1t", [2 * F, H], BF16, kind="ExternalInput")
    w2t = nc.dram_tensor("w2t", [2 * H, H], BF16, kind="ExternalInput")
    wct = nc.dram_tensor("wct", [H, C], BF16, kind="ExternalInput")
    outd = nc.dram_tensor("out", [D0SH, C], F32, kind="ExternalOutput")

    rg = [list(range(NCORES))]
    relu = mybir.ActivationFunctionType.Relu
    DR = mybir.MatmulPerfMode.DoubleRow

    with tile.TileContext(nc) as tc:
        with (
            tc.tile_pool(name="const", bufs=1) as constp,
            tc.tile_pool(name="stream", bufs=6) as streamp,
            tc.tile_pool(name="stage", bufs=4) as stagep,
            tc.tile_pool(name="h1p", bufs=3) as h1p,
            tc.tile_pool(name="ps1p", bufs=2, space="PSUM") as ps1p,
            tc.tile_pool(name="ps0p", bufs=1, space="PSUM") as ps0p,
            tc.tile_pool(name="ps2p", bufs=2, space="PSUM") as ps2p,
            tc.tile_pool(name="ps34p", bufs=1, space="PSUM") as ps34p,
            tc.tile_pool(name="dram", bufs=1, space="DRAM") as dramp,
        ):
            S_sb = constp.tile([128, KT, F], FP8E4, name="S_sb")
            dfT_sb = constp.tile([F, KP], BF16, name="dfT_sb")
            d0e_sb = constp.tile([128, KT, 2 * H], FP8E4, name="d0e_sb")
            w1_sb = constp.tile([128, 2, H], BF16, name="w1_sb")
            w2_sb = constp.tile([128, 2, H], BF16, name="w2_sb")
            wc_sb = constp.tile([H, C], BF16, name="wc_sb")
            ars_sb = constp.tile([F, KP], BF16, name="ars_sb")

            # stream-critical constant first; the rest on the gpsimd (SWDGE)
            # ring so they never delay the first stream slab
            nc.sync.dma_start(S_sb[:], sfp.ap())
            nc.gpsimd.dma_start(d0e_sb[:], d0ep.ap())
            nc.gpsimd.dma_start(dfT_sb[:], dfT.ap())
            nc.gpsimd.dma_start(
                w1_sb[:], w1t.ap().rearrange("(c p) e -> p c e", p=128)
            )
            nc.gpsimd.dma_start(
                w2_sb[:], w2t.ap().rearrange("(c p) e -> p c e", p=128)
            )
            nc.gpsimd.dma_start(wc_sb[:], wct.ap())
            # the dense loop reads h1 rows 6000..6016 whose agg columns are
            # never streamed; zero them so no NaN garbage flows through relu
            # (their d0e rows are zero, but NaN * 0 = NaN)
            nc.vector.memset(ars_sb[:, D1:KP], 0.0)

            # ---- big streamed matmul: agg1T partial [128, 6000] ----
            # (`repeat` re-runs the whole pipeline for benchmark slope timing;
            # results are identical each rep so output is unchanged)
            for _rep in range(repeat):
              ar_ins, ar_outs = [], []
              for g, (_, _, goff, gw) in enumerate(ARG):
                ai = dramp.tile([F, gw], BF16, name=f"ar_in{_rep}_{g}")
                ao = dramp.tile(
                    [F, gw], BF16, name=f"ar_out{_rep}_{g}", addr_space="Shared"
                )
                ar_ins.append(ai)
                ar_outs.append(ao)
              for j in range(JB):
                w = JW if j < JB - 1 else JWL
                # two half-slab DMAs per dst block on the two HWDGE rings so
                # the first matmul only waits on half the 3 MB transfer
                ra = streamp.tile([128, HKT, JW], FP8E4, tag="ra")
                rb = streamp.tile([128, KT - HKT, JW], FP8E4, tag="rb")
                if j < JB - 1:
                    nc.sync.dma_start(ra[:], difp.ap()[j, :, 0:HKT, :])
                    nc.scalar.dma_start(rb[:], difp.ap()[j, :, HKT:KT, :])
                else:
                    nc.sync.dma_start(
                        ra[:, :, 0:JWL], difl.ap()[:, 0:HKT, :]
                    )
                    nc.scalar.dma_start(
                        rb[:, :, 0:JWL], difl.ap()[:, HKT:KT, :]
                    )
                ps1 = ps1p.tile([F, JW], F32, tag="ps1")
                for p in range(HKT // 2):
                    nc.tensor.matmul(
                        ps1[:, 0:w],
                        S_sb[:, 2 * p : 2 * p + 2, :],
                        ra[:, 2 * p : 2 * p + 2, 0:w],
                        start=(p == 0),
                        stop=False,
                        perf_mode=DR,
                    )
                for p in range(HKT // 2, KT // 2):
                    q = 2 * p - HKT
                    nc.tensor.matmul(
                        ps1[:, 0:w],
                        S_sb[:, 2 * p : 2 * p + 2, :],
                        rb[:, q : q + 2, 0:w],
                        start=False,
                        stop=False,
                        perf_mode=DR,
                    )
                nc.tensor.matmul(
                    ps1[:, 0:w],
                    S_sb[:, KT - 1, :],
                    rb[:, KT - 1 - HKT, 0:w],
                    start=False,
                    stop=True,
                )
                st = stagep.tile([F, JW], BF16, tag="st")
                nc.vector.tensor_scalar_mul(st[:, 0:w], ps1[:, 0:w], ISCALE)
                g = next(i for i, a in enumerate(ARG) if a[0] <= j < a[1])
                off = j * JW - ARG[g][2]
                nc.gpsimd.dma_start(
                    ar_ins[g][:, off : off + w], st[:, 0:w]
                )
                if j == ARG[g][1] - 1:
                    nc.gpsimd.collective_compute(
                        "AllReduce",
                        mybir.AluOpType.add,
                        replica_groups=rg,
                        ins=[ar_ins[g].opt()],
                        outs=[ar_outs[g].opt()],
                    )
              # summed slices come back on the sync ring, after the stream
              for g, (_, _, goff, gw) in enumerate(ARG):
                nc.sync.dma_start(
                    ars_sb[:, goff : goff + gw], ar_outs[g][:]
                )

              # ---- level-1 dense + level-0 aggregation, fused per h1 chunk ----
              ps0 = ps0p.tile([H, 2 * H], F32, name="ps0")
              for t in range(KT):
                ps2 = ps2p.tile([128, H], F32, tag="ps2")
                nc.tensor.matmul(
                    ps2[:],
                    ars_sb[:, t * 128 : (t + 1) * 128],
                    w1_sb[:, 0, :],
                    start=True,
                    stop=False,
                )
                nc.tensor.matmul(
                    ps2[:],
                    dfT_sb[:, t * 128 : (t + 1) * 128],
                    w1_sb[:, 1, :],
                    start=False,
                    stop=True,
                )
                h1t = h1p.tile([128, H], BF16, tag="h1t")
                nc.scalar.activation(h1t[:], ps2[:], relu)
                nc.tensor.matmul(
                    ps0[:],
                    h1t[:],
                    d0e_sb[:, t, :],
                    start=(t == 0),
                    stop=(t == KT - 1),
                )

            # ---- level-0 dense + classifier + softmax ----
            c0_sb = constp.tile([H, 2 * H], BF16, name="c0_sb")
            nc.vector.tensor_scalar_mul(c0_sb[:], ps0[:], ISC0)
            ps3 = ps34p.tile([H, D0SH], F32, tag="ps34")
            nc.tensor.matmul(
                ps3[:], w2_sb[:, 0, :], c0_sb[:, 0:H], start=True, stop=False
            )
            nc.tensor.matmul(
                ps3[:], w2_sb[:, 1, :], c0_sb[:, H : 2 * H], start=False, stop=True
            )
            h0T = constp.tile([H, D0SH], BF16, name="h0T")
            nc.scalar.activation(h0T[:], ps3[:], relu)
            ps4 = ps34p.tile([D0SH, C], F32, tag="ps34")
            nc.tensor.matmul(ps4[:], h0T[:], wc_sb[:], start=True, stop=True)

            mx = constp.tile([D0SH, 1], F32, name="mx")
            nc.vector.tensor_reduce(
                mx[:], ps4[:], axis=mybir.AxisListType.X, op=mybir.AluOpType.max
            )
            nmx = constp.tile([D0SH, 1], F32, name="nmx")
            nc.vector.tensor_scalar_mul(nmx[:], mx[:], -1.0)
            esb = constp.tile([D0SH, C], F32, name="esb")
            ssum = constp.tile([D0SH, 1], F32, name="ssum")
            nc.scalar.activation(
                esb[:],
                ps4[:],
                mybir.ActivationFunctionType.Exp,
                bias=nmx[:],
                accum_out=ssum[:],
            )
            rs = constp.tile([D0SH, 1], F32, name="rs")
            nc.vector.reciprocal(rs[:], ssum[:])
            osb = constp.tile([D0SH, C], F32, name="osb")
            nc.vector.tensor_scalar_mul(osb[:], esb[:], rs[:])
            nc.sync.dma_start(outd.ap(), osb[:])

    nc.compile()
    return nc


def _prep_in_maps(
    features,
    src_nodes,
    dst_idx_1,
    src_idx_1,
    dif_mat_1,
    dst_idx_0,
    src_idx_0,
    dif_mat_0,
    w1,
    w2,
    w_cls,
):
    f32 = np.float32
    bf16 = ml_dtypes.bfloat16
    fp8 = ml_dtypes.float8_e4m3
    features = np.asarray(features, f32)
    dif_mat_1 = np.asarray(dif_mat_1, f32)
    dif_mat_0 = np.asarray(dif_mat_0, f32)
    src_nodes = np.asarray(src_nodes)
    gsrc = src_nodes[np.asarray(src_idx_1)]  # [48000] rows into features
    gdst = src_nodes[np.asarray(dst_idx_1)]  # [6000]

    dfT = np.zeros((F, KP), f32)
    dfT[:, :D1] = features[gdst].T

    difT0exp = np.zeros((KP, D0), f32)
    np.add.at(difT0exp, np.asarray(src_idx_0), dif_mat_0.T)
    E = np.zeros((KP, D0), f32)
    E[np.asarray(dst_idx_0), np.arange(D0)] = 1.0

    w1c = np.ascontiguousarray(w1).astype(bf16)
    w2c = np.ascontiguousarray(w2).astype(bf16)
    wcc = np.ascontiguousarray(w_cls).astype(bf16)
    dfT16 = dfT.astype(bf16)

    in_maps = []
    for k in range(NCORES):
        sl = slice(k * CH, (k + 1) * CH)
        # fp8 stream, packed [JB-1, 128, KT, JW] + narrow last block:
        #   difp[j, p, kt, e] = dif[src = kt*128+p, dst = j*512+e] * SCALE
        P = np.zeros((KP, D1), f32)
        P[:CH, :] = dif_mat_1[:, sl].T
        Q = (P * SCALE).astype(fp8)                      # [KP, 6000]
        full = Q[:, : (JB - 1) * JW].reshape(KT, 128, JB - 1, JW)
        difp = np.ascontiguousarray(full.transpose(2, 1, 0, 3))
        difl = np.ascontiguousarray(
            Q[:, (JB - 1) * JW :].reshape(KT, 128, JWL).transpose(1, 0, 2)
        )

        # src features (unscaled fp8), packed [128, KT, F]
        sf = np.zeros((KP, F), f32)
        sf[:CH] = features[gsrc[sl]]
        sfp = np.ascontiguousarray(
            sf.astype(fp8).reshape(KT, 128, F).transpose(1, 0, 2)
        )

        # [difT0exp | E] columns for this core's targets, packed [128, KT, 2H]
        d0e = np.zeros((KP, 2 * H), f32)
        d0e[:, :H] = difT0exp[:, k * D0SH : (k + 1) * D0SH]
        d0e[:, H:] = E[:, k * D0SH : (k + 1) * D0SH]
        d0ep = np.ascontiguousarray(
            (d0e * SC0).astype(fp8).reshape(KT, 128, 2 * H).transpose(1, 0, 2)
        )

        in_maps.append(
            {
                "difp": difp,
                "difl": difl,
                "sfp": sfp,
                "dfT": dfT16,
                "d0ep": d0ep,
                "w1t": w1c,
                "w2t": w2c,
                "wct": wcc,
            }
        )
    return in_maps


def kernel(**inputs):
    global _nc, LAST
    if _nc is None:
        _nc = _build()
    in_maps = _prep_in_maps(**inputs)
    res = bass_utils.run_bass_kernel_spmd(
        _nc,
        in_maps,
        core_ids=list(range(NCORES)),
        trace=TRACE,
        **TRACE_KW,
    )
    LAST = res
    out = np.concatenate([res.results[k]["out"] for k in range(NCORES)], axis=0)
    return out.astype(np.float32)
